# revision 1
# baseline (speedup 1.0000x reference)
"""Trainium2 Bass kernel for DecomposingAttnProcessor (pad variant).

Math (pad branch contributes exactly zero since pad tokens are zeros
projected with no bias -> k_pad = v_pad = 0):
    q = hs @ Wq.T / (temp + eps)   (scale folded into Wq on host)
    k = ehs @ Wk.T ; v = ehs @ Wv.T
    scores[c,h,s,e] = q . k        (per head, dh=64)
    w = softmax over the 4 components c (dim 0)
    o = w @ v ; out = o @ Wo.T + bo + hs

Sharding: 8 cores, split S=4096 into 512-row blocks; all 4 components of
a block stay on one core (softmax couples them). K/V computed redundantly
per core (encoder seq is only 154).

Device layout is fully transposed (features on partitions): inputs are
host-transposed, output is produced transposed and host-untransposed.
"""

import numpy as np
import ml_dtypes

import concourse.bass as bass
import concourse.mybir as mybir
import concourse.tile as tile
from concourse import bacc
from concourse.bass_utils import run_bass_kernel_spmd

F32 = mybir.dt.float32
F32R = mybir.dt.float32r
BF16 = mybir.dt.bfloat16
AF = mybir.ActivationFunctionType
ALU = mybir.AluOpType

NCOMP = 4
HEADS = 24
DH = 64
D = 1536
S = 4096
E = 154
EPS = 1e-8
NCORES = 8
SL = S // NCORES          # 512 s-rows per core (per component)
SH = SL // 2              # 256: s-half tile width (N of most matmuls)
FT = D // 128             # 12 feature tiles of 128
HP = HEADS // 2           # 12 head-pairs (2 heads = 128 feature rows)
ECAT = NCOMP * E          # 616: components stacked along encoder axis
ETILES = ((0, 128), (128, E - 128))   # e split: 128 + 26


def _emit(tc):
    import os
    phases = os.environ.get("K_PHASES", "ABC")
    blevel = int(os.environ.get("K_BLEVEL", "4"))
    nc = tc.nc

    xT = nc.declare_dram_parameter("xT", [NCOMP, D, SL], F32, isOutput=False)
    xTb = nc.declare_dram_parameter("xTb", [NCOMP, D, SL], BF16, isOutput=False)
    eT = nc.declare_dram_parameter("eT", [D, ECAT], BF16, isOutput=False)
    wqT = nc.declare_dram_parameter("wqT", [D, D], BF16, isOutput=False)
    wkT = nc.declare_dram_parameter("wkT", [D, D], BF16, isOutput=False)
    wvT = nc.declare_dram_parameter("wvT", [D, D], BF16, isOutput=False)
    woT = nc.declare_dram_parameter("woT", [D, D], BF16, isOutput=False)
    bo = nc.declare_dram_parameter("bo", [128, FT], F32, isOutput=False)
    outT = nc.declare_dram_parameter("outT", [NCOMP, D, SL], F32, isOutput=True)

    # DRAM views with the 128-row tile index folded into the free dim, so a
    # whole [1536, n] panel loads as one DMA into a [128, FT*n] tile.
    xT_v = [xT[c].rearrange("(f p) s -> p f s", p=128) for c in range(NCOMP)]
    xTb_v = [xTb[c].rearrange("(f p) s -> p f s", p=128) for c in range(NCOMP)]
    eT_v = eT.rearrange("(f p) e -> p f e", p=128)
    wqT_v = wqT.rearrange("(f p) o -> p f o", p=128)
    wkT_v = wkT.rearrange("(f p) o -> p f o", p=128)
    wvT_v = wvT.rearrange("(f p) o -> p f o", p=128)
    woT_v = woT.rearrange("(f p) o -> p f o", p=128)
    outT_v = [outT[c].rearrange("(f p) s -> p f s", p=128) for c in range(NCOMP)]

    with tc.tile_pool(name="persist", bufs=1) as pp:
        # ---------------- persistent tiles ----------------
        kt_sb = [pp.tile([128, ECAT], BF16, tag="kT", bufs=FT, name=f"kt{t}")
                 for t in range(FT)]
        v_sb = [[pp.tile([esz, D], BF16, tag=f"v{ei}", bufs=NCOMP,
                         name=f"v{c}_{ei}")
                 for ei, (eo, esz) in enumerate(ETILES)] for c in range(NCOMP)]
        bo_sb = pp.tile([128, FT], F32, tag="bo", bufs=1, name="bo_sb")
        nc.sync.dma_start(out=bo_sb[:], in_=bo[:])

        def _phases():
            # ---------------- phase A: K^T and V ----------------
            if "A" in phases:
              with (
                tc.tile_pool(name="pha", bufs=1) as pa,
                tc.tile_pool(name="pha_psum", bufs=1, space="PSUM") as pap,
              ):
                et_b = pa.tile([128, FT * ECAT], BF16, tag="eT", bufs=1,
                               name="et_b")
                nc.sync.dma_start(
                    out=et_b.rearrange("p (f e) -> p f e", f=FT), in_=eT_v)
                et = [et_b[:, fi * ECAT:(fi + 1) * ECAT] for fi in range(FT)]

                # K^T[fo, c*E + e] over fi; N split 308+308 (>=256 keeps f32r
                # at full rate)
                for fot in range(FT):
                    wk_b = pa.tile([128, FT * 128], BF16, tag="wk", bufs=3,
                                   name=f"wk{fot}")
                    nc.sync.dma_start(
                        out=wk_b.rearrange("p (f o) -> p f o", f=FT),
                        in_=wkT_v[:, :, fot * 128:(fot + 1) * 128])
                    for nch in range(2):
                        n0 = nch * 308
                        pk = pap.tile([128, 308], F32, tag="pk", bufs=2,
                                      name=f"pk{fot}_{nch}")
                        for fi in range(FT):
                            nc.tensor.matmul(
                                pk[:], wk_b[:, fi * 128:(fi + 1) * 128],
                                et[fi][:, n0:n0 + 308],
                                start=(fi == 0), stop=(fi == FT - 1))
                        nc.vector.tensor_copy(
                            out=kt_sb[fot][:, n0:n0 + 308], in_=pk[:])

                # V[c][e, fv] (natural layout, bf16) over fi
                for fvc in range(3):
                    wv_b = pa.tile([128, FT * 512], BF16, tag="wv", bufs=2,
                                   name=f"wv{fvc}")
                    nc.sync.dma_start(
                        out=wv_b.rearrange("p (f o) -> p f o", f=FT),
                        in_=wvT_v[:, :, fvc * 512:(fvc + 1) * 512])
                    for c in range(NCOMP):
                        for ei, (eo, esz) in enumerate(ETILES):
                            pv = pap.tile([128, 512], F32, tag="pv", bufs=2,
                                          name=f"pv{fvc}_{c}_{ei}")
                            for fi in range(FT):
                                nc.tensor.matmul(
                                    pv[:esz, :],
                                    et[fi][:, c * E + eo:c * E + eo + esz],
                                    wv_b[:, fi * 512:(fi + 1) * 512],
                                    start=(fi == 0), stop=(fi == FT - 1))
                            nc.vector.tensor_copy(
                                out=v_sb[c][ei][:, fvc * 512:(fvc + 1) * 512],
                                in_=pv[:esz, :])

            # ---------------- phases B+C per s-half ----------------
            with (
                tc.tile_pool(name="bc", bufs=1) as bc,
                tc.tile_pool(name="bcp", bufs=1, space="PSUM") as bcp,
            ):
                for half in range(2):
                    s0 = half * SH
                    # bf16 x^T panels for the Q projection
                    xh = []
                    for c in range(NCOMP):
                        t = bc.tile([128, FT * SH], BF16, tag="xh", bufs=5,
                                    name=f"xh{half}_{c}")
                        nc.sync.dma_start(
                            out=t.rearrange("p (f s) -> p f s", f=FT),
                            in_=xTb_v[c][:, :, s0:s0 + SH])
                        xh.append(t)

                    # -------- phase B: Q, scores, softmax, o --------
                    ot_sb = {}
                    for hp in range(HP if "B" in phases else 0):
                        wq_b = bc.tile([128, FT * 128], BF16, tag="wq", bufs=2,
                                       name=f"wq{half}_{hp}")
                        nc.sync.dma_start(
                            out=wq_b.rearrange("p (f o) -> p f o", f=FT),
                            in_=wqT_v[:, :, hp * 128:(hp + 1) * 128])

                        # Q^T for the two heads of this pair, all 4 components
                        qt = []
                        for c in range(NCOMP):
                            pq = bcp.tile([128, SH], F32, tag="pq", bufs=2,
                                          name=f"pq{half}_{hp}_{c}")
                            for fi in range(FT):
                                nc.tensor.matmul(
                                    pq[:], wq_b[:, fi * 128:(fi + 1) * 128],
                                    xh[c][:, fi * SH:(fi + 1) * SH],
                                    start=(fi == 0), stop=(fi == FT - 1))
                            q = bc.tile([128, SH], BF16, tag="qT", bufs=6,
                                        name=f"qt{half}_{hp}_{c}")
                            nc.scalar.copy(q[:], pq[:])
                            qt.append(q)
                        if blevel < 2:
                            continue

                        # scores + exp, both heads packed along the free dim
                        exps = [[None, None] for _ in range(NCOMP)]
                        for ei, (eo, esz) in enumerate(ETILES):
                            for c in range(NCOMP):
                                # separate psum banks per head: matmul psum
                                # writes must start at a bank boundary
                                ex = bc.tile([esz, 2 * SH], BF16, tag=f"exp{ei}",
                                             bufs=6, name=f"ex{half}_{hp}_{ei}_{c}")
                                for hh in range(2):
                                    ps = bcp.tile([128, SH], F32, tag="ps",
                                                  bufs=3,
                                                  name=f"ps{half}_{hp}_{ei}_{c}_{hh}")
                                    nc.tensor.matmul(
                                        ps[:esz, :],
                                        kt_sb[hp][hh * 64:(hh + 1) * 64,
                                                  c * E + eo:c * E + eo + esz],
                                        qt[c][hh * 64:(hh + 1) * 64, :],
                                        start=True, stop=True)
                                    nc.scalar.activation(
                                        ex[:, hh * SH:(hh + 1) * SH],
                                        ps[:esz, :], AF.Exp)
                                exps[c][ei] = ex
                            ssum = bc.tile([esz, 2 * SH], BF16, tag=f"sum{ei}",
                                           bufs=4, name=f"sm{half}_{hp}_{ei}")
                            nc.vector.tensor_add(out=ssum[:], in0=exps[0][ei][:],
                                                 in1=exps[1][ei][:])
                            nc.vector.tensor_add(out=ssum[:], in0=ssum[:],
                                                 in1=exps[2][ei][:])
                            nc.vector.tensor_add(out=ssum[:], in0=ssum[:],
                                                 in1=exps[3][ei][:])
                            rinv = bc.tile([esz, 2 * SH], BF16, tag=f"sum{ei}",
                                           bufs=4, name=f"ri{half}_{hp}_{ei}")
                            with nc.allow_low_precision(
                                    reason="softmax weights are consumed in bf16"):
                                nc.vector.reciprocal(out=rinv[:], in_=ssum[:])
                            for c in range(NCOMP):
                                w = bc.tile([esz, 2 * SH], BF16, tag=f"w{ei}",
                                            bufs=6, name=f"w{half}_{hp}_{ei}_{c}")
                                nc.vector.tensor_mul(out=w[:], in0=exps[c][ei][:],
                                                     in1=rinv[:])
                                exps[c][ei] = w  # normalized weights

                        # o^T: V-slices @ w; head hh lands on psum partitions
                        # hh*64..hh*64+64 (own accumulation group per head, both
                        # column-aligned to the bank start)
                        for c in range(NCOMP if blevel >= 4 else 0):
                            po = bcp.tile([128, SH], F32, tag="po", bufs=2,
                                          name=f"po{half}_{hp}_{c}")
                            for hh in range(2):
                                h = hp * 2 + hh
                                for ei, (eo, esz) in enumerate(ETILES):
                                    nc.tensor.matmul(
                                        po[hh * 64:(hh + 1) * 64, :],
                                        v_sb[c][ei][:, h * 64:(h + 1) * 64],
                                        exps[c][ei][:, hh * SH:(hh + 1) * SH],
                                        start=(ei == 0), stop=(ei == 1),
                                        skip_group_check=True)
                            ot = bc.tile([128, SH], BF16, tag="oT", bufs=48,
                                         name=f"ot{half}_{hp}_{c}")
                            nc.vector.tensor_copy(out=ot[:], in_=po[:])
                            ot_sb[(c, hp)] = ot

                    # -------- phase C: out-proj + bias + residual --------
                    for fot in range(FT if "C" in phases else 0):
                        wo_b = bc.tile([128, FT * 128], BF16, tag="wo", bufs=3,
                                       name=f"wo{half}_{fot}")
                        nc.sync.dma_start(
                            out=wo_b.rearrange("p (f o) -> p f o", f=FT),
                            in_=woT_v[:, :, fot * 128:(fot + 1) * 128])
                        for c in range(NCOMP):
                            xr = bc.tile([128, SH], F32, tag="xr", bufs=4,
                                         name=f"xr{half}_{fot}_{c}")
                            nc.sync.dma_start(
                                out=xr[:],
                                in_=xT_v[c][:, fot, s0:s0 + SH])
                            po = bcp.tile([128, SH], F32, tag="pout", bufs=1,
                                          name=f"pc{half}_{fot}_{c}")
                            for fi in range(FT):
                                nc.tensor.matmul(
                                    po[:], wo_b[:, fi * 128:(fi + 1) * 128],
                                    ot_sb[(c, fi)][:],
                                    start=(fi == 0), stop=(fi == FT - 1))
                            ob = bc.tile([128, SH], F32, tag="outsb", bufs=4,
                                         name=f"ob{half}_{fot}_{c}")
                            nc.vector.scalar_tensor_tensor(
                                out=ob[:], in0=po[:],
                                scalar=bo_sb[:, fot:fot + 1],
                                in1=xr[:],
                                op0=ALU.add, op1=ALU.add)
                            nc.sync.dma_start(
                                out=outT_v[c][:, fot, s0:s0 + SH], in_=ob[:])


        repeat = int(os.environ.get("K_REPEAT", "1"))
        for _rep in range(repeat):
            _phases()


_NC_CACHE = {}


def _get_nc():
    if "nc" not in _NC_CACHE:
        nc = bacc.Bacc("TRN2", target_bir_lowering=False)
        with tile.TileContext(nc) as tc:
            _emit(tc)
        nc.compile()
        _NC_CACHE["nc"] = nc
    return _NC_CACHE["nc"]


def kernel(hidden_states, encoder_hidden_states, temperature, Wq, Wk, Wv, Wo,
           bo, pad_length):
    # pad branch contributes zero to the output (zeros projected with no
    # bias give k_pad = v_pad = 0), so pad_length is irrelevant.
    hs = np.ascontiguousarray(np.asarray(hidden_states, dtype=np.float32))
    ehs = np.ascontiguousarray(
        np.asarray(encoder_hidden_states, dtype=np.float32))
    temp = float(np.asarray(temperature).reshape(-1)[0])
    Wq = np.asarray(Wq, dtype=np.float32)
    Wk = np.asarray(Wk, dtype=np.float32)
    Wv = np.asarray(Wv, dtype=np.float32)
    Wo = np.asarray(Wo, dtype=np.float32)
    bo_v = np.asarray(bo, dtype=np.float32).reshape(-1)

    wqT = np.ascontiguousarray((Wq / (temp + EPS)).T).astype(ml_dtypes.bfloat16)
    wkT = np.ascontiguousarray(Wk.T).astype(ml_dtypes.bfloat16)
    wvT = np.ascontiguousarray(Wv.T).astype(ml_dtypes.bfloat16)
    woT = np.ascontiguousarray(Wo.T).astype(ml_dtypes.bfloat16)
    eT_all = np.ascontiguousarray(
        np.concatenate([ehs[c].T for c in range(NCOMP)],
                       axis=1)).astype(ml_dtypes.bfloat16)
    bo_t = np.ascontiguousarray(bo_v.reshape(FT, 128).T)

    nc = _get_nc()
    in_maps = []
    for i in range(NCORES):
        xT_i = np.ascontiguousarray(
            hs[:, i * SL:(i + 1) * SL, :].transpose(0, 2, 1))
        in_maps.append({
            "xT": xT_i, "xTb": xT_i.astype(ml_dtypes.bfloat16),
            "eT": eT_all, "wqT": wqT, "wkT": wkT,
            "wvT": wvT, "woT": woT, "bo": bo_t,
        })

    res = run_bass_kernel_spmd(nc, in_maps, core_ids=list(range(NCORES)))

    out = np.empty((NCOMP, S, D), dtype=np.float32)
    for i in range(NCORES):
        out[:, i * SL:(i + 1) * SL, :] = res.results[i]["outT"].transpose(
            0, 2, 1)
    return out



# revision 6
# speedup vs baseline: 1.8532x; 1.8532x over previous
"""Trainium2 Bass kernel for DecomposingAttnProcessor (pad variant).

Math (pad branch contributes exactly zero since pad tokens are zeros
projected with no bias -> k_pad = v_pad = 0):
    q = hs @ Wq.T / (temp + eps)   (scale folded into Wq on host)
    k = ehs @ Wk.T ; v = ehs @ Wv.T
    scores[c,h,s,e] = q . k        (per head, dh=64)
    w = softmax over the 4 components c (dim 0)
    o = w @ v ; out = o @ Wo.T + bo + hs

Sharding: 8 cores, split S=4096 into 512-row blocks; all 4 components of
a block stay on one core (softmax couples them). K/V computed redundantly
per core (encoder seq is only 154).

Layout notes:
- Device layout fully transposed (features on partitions).
- E padded to 192 host-side with zeros: padded e slots give scores=0 ->
  exp(0)=1 -> w=1/4, but v rows there are 0, so output unaffected.
- Per component: e split (0:128) in v_sb[c]; the 64-row e-tails are packed
  two components per tile (partition offsets 0/64): vpad[0]=(c0,c1),
  vpad[1]=(c2,c3). Same packing for the tail scores/weights (ex1 tiles).
- Softmax over the 4 components: e-main via 3 DVE adds; e-tail comp-sum
  via a stacked-identity matmul on PE (result replicated to both offsets).
- Single s-pass of 512 (N=512 matmuls), weights streamed once; residual
  comes from the bf16 x panels already in SBUF.
"""

import numpy as np
import ml_dtypes

import concourse.bass as bass
import concourse.mybir as mybir
import concourse.tile as tile
from concourse import bacc
from concourse.bass_utils import run_bass_kernel_spmd

F32 = mybir.dt.float32
BF16 = mybir.dt.bfloat16
AF = mybir.ActivationFunctionType
ALU = mybir.AluOpType

NCOMP = 4
HEADS = 24
DH = 64
D = 1536
S = 4096
E = 154
EP = 192                  # e padded so the tail is 64 rows (offsets 0/64 only)
ECAT = NCOMP * EP         # 768
EPS = 1e-8
NCORES = 8
SL = S // NCORES          # 512 s-rows per core (per component)
FT = D // 128             # 12 feature tiles of 128
HP = HEADS // 2           # 12 head-pairs (2 heads = 128 feature rows)


def _emit(tc):
    import os
    phases = os.environ.get("K_PHASES", "ABC")
    nc = tc.nc

    xTb = nc.declare_dram_parameter("xTb", [NCOMP, D, SL], BF16, isOutput=False)
    eT = nc.declare_dram_parameter("eT", [D, ECAT], BF16, isOutput=False)
    wqT = nc.declare_dram_parameter("wqT", [D, D], BF16, isOutput=False)
    wkT = nc.declare_dram_parameter("wkT", [D, D], BF16, isOutput=False)
    wvT = nc.declare_dram_parameter("wvT", [D, D], BF16, isOutput=False)
    woT = nc.declare_dram_parameter("woT", [D, D], BF16, isOutput=False)
    bo = nc.declare_dram_parameter("bo", [128, FT], F32, isOutput=False)
    ist = nc.declare_dram_parameter("ist", [128, 128], BF16, isOutput=False)
    outT = nc.declare_dram_parameter("outT", [NCOMP, D, SL], F32, isOutput=True)

    # DRAM views with the 128-row tile index folded into the free dim, so a
    # whole [1536, n] panel loads as one DMA into a [128, FT*n] tile.
    xTb_v = [xTb[c].rearrange("(f p) s -> p f s", p=128) for c in range(NCOMP)]
    eT_v = eT.rearrange("(f p) e -> p f e", p=128)
    wqT_v = wqT.rearrange("(f p) o -> p f o", p=128)
    wkT_v = wkT.rearrange("(f p) o -> p f o", p=128)
    wvT_v = wvT.rearrange("(f p) o -> p f o", p=128)
    woT_v = woT.rearrange("(f p) o -> p f o", p=128)
    outT_v = [outT[c].rearrange("(f p) s -> p f s", p=128) for c in range(NCOMP)]

    with tc.tile_pool(name="persist", bufs=1) as pp:
        # ---------------- persistent tiles ----------------
        kt_sb = [pp.tile([128, ECAT], BF16, tag="kT", bufs=FT, name=f"kt{t}")
                 for t in range(FT)]
        v_sb = [pp.tile([128, D], BF16, tag="v", bufs=NCOMP, name=f"v{c}")
                for c in range(NCOMP)]
        vpad = [pp.tile([128, D], BF16, tag="vpad", bufs=2, name=f"vpad{g}")
                for g in range(2)]
        bo_sb = pp.tile([128, FT], F32, tag="bo", bufs=1, name="bo_sb")
        ist_sb = pp.tile([128, 128], BF16, tag="ist", bufs=1, name="ist_sb")
        xh = [pp.tile([128, FT * SL], BF16, tag="xh", bufs=NCOMP, name=f"xh{c}")
              for c in range(NCOMP)]
        nc.sync.dma_start(out=bo_sb[:], in_=bo[:])
        nc.sync.dma_start(out=ist_sb[:], in_=ist[:])

        # ---------------- phase A: K^T and V ----------------
        with (
            tc.tile_pool(name="pha", bufs=1) as pa,
            tc.tile_pool(name="pap", bufs=1, space="PSUM") as pap,
        ):
            et_b = pa.tile([128, FT * ECAT], BF16, tag="eT", bufs=1, name="et_b")
            nc.sync.dma_start(
                out=et_b.rearrange("p (f e) -> p f e", f=FT), in_=eT_v)
            # x panels needed at phase-B start; issue early so they overlap A.
            for c in range(NCOMP):
                nc.sync.dma_start(
                    out=xh[c].rearrange("p (f s) -> p f s", f=FT),
                    in_=xTb_v[c])
            et = et_b.rearrange("p (f e) -> p f e", f=FT)       # [128,12,768]

            if "A" in phases:
                # K^T[fo, c*EP + e] accumulated over fi; N split 512+256.
                for fot in range(FT):
                    wk_b = pa.tile([128, FT * 128], BF16, tag="wk", bufs=3,
                                   name=f"wk{fot}")
                    nc.sync.dma_start(
                        out=wk_b.rearrange("p (f o) -> p f o", f=FT),
                        in_=wkT_v[:, :, fot * 128:(fot + 1) * 128])
                    pk0 = pap.tile([128, 512], F32, tag="pk0", bufs=2,
                                   name=f"pk0_{fot}")
                    pk1 = pap.tile([128, 256], F32, tag="pk1", bufs=2,
                                   name=f"pk1_{fot}")
                    for fi in range(FT):
                        w_sl = wk_b[:, fi * 128:(fi + 1) * 128]
                        nc.tensor.matmul(pk0[:], w_sl, et[:, fi, 0:512],
                                         start=(fi == 0), stop=(fi == FT - 1))
                        nc.tensor.matmul(pk1[:], w_sl, et[:, fi, 512:768],
                                         start=(fi == 0), stop=(fi == FT - 1))
                    nc.vector.tensor_copy(out=kt_sb[fot][:, 0:512], in_=pk0[:])
                    nc.vector.tensor_copy(out=kt_sb[fot][:, 512:768], in_=pk1[:])

                # V[c][e, fv] (natural layout, bf16) accumulated over fi.
                for fvc in range(3):
                    wv_b = pa.tile([128, FT * 512], BF16, tag="wv", bufs=2,
                                   name=f"wv{fvc}")
                    nc.sync.dma_start(
                        out=wv_b.rearrange("p (f o) -> p f o", f=FT),
                        in_=wvT_v[:, :, fvc * 512:(fvc + 1) * 512])
                    for c in range(NCOMP):
                        pv = pap.tile([128, 512], F32, tag="pv", bufs=2,
                                      name=f"pv{fvc}_{c}")
                        for fi in range(FT):
                            nc.tensor.matmul(
                                pv[:],
                                et[:, fi, c * 128:c * 128 + 128],
                                wv_b[:, fi * 512:(fi + 1) * 512],
                                start=(fi == 0), stop=(fi == FT - 1))
                        nc.vector.tensor_copy(
                            out=v_sb[c][:, fvc * 512:(fvc + 1) * 512], in_=pv[:])
                    # e-tail rows, two comps per tile at offsets 0/64.
                    for g in range(2):
                        pvp = pap.tile([128, 512], F32, tag="pv", bufs=2,
                                       name=f"pvp{fvc}_{g}")
                        for fi in range(FT):
                            nc.tensor.matmul(
                                pvp[:],
                                et[:, fi, 512 + g * 128:512 + g * 128 + 128],
                                wv_b[:, fi * 512:(fi + 1) * 512],
                                start=(fi == 0), stop=(fi == FT - 1))
                        nc.vector.tensor_copy(
                            out=vpad[g][:, fvc * 512:(fvc + 1) * 512],
                            in_=pvp[:])

        # ---------------- phases B+C ----------------
        with (
            tc.tile_pool(name="bc", bufs=1) as bc,
            tc.tile_pool(name="bcp", bufs=1, space="PSUM") as bcp,
        ):
            ot_sb = {}

            def emit_q(hp):
                wq_b = bc.tile([128, FT * 128], BF16, tag="wq", bufs=2,
                               name=f"wq{hp}")
                nc.sync.dma_start(
                    out=wq_b.rearrange("p (f o) -> p f o", f=FT),
                    in_=wqT_v[:, :, hp * 128:(hp + 1) * 128])
                qt = []
                for pair in range(2):
                    cs = (2 * pair, 2 * pair + 1)
                    pq = {c: bcp.tile([128, SL], F32, tag="pq", bufs=2,
                                      name=f"pq{hp}_{c}") for c in cs}
                    for fi in range(FT):
                        w_sl = wq_b[:, fi * 128:(fi + 1) * 128]
                        for c in cs:
                            nc.tensor.matmul(
                                pq[c][:], w_sl, xh[c][:, fi * SL:(fi + 1) * SL],
                                start=(fi == 0), stop=(fi == FT - 1))
                    for c in cs:
                        q = bc.tile([128, SL], BF16, tag="qT", bufs=6,
                                    name=f"qt{hp}_{c}")
                        nc.vector.tensor_copy(out=q[:], in_=pq[c][:])
                        qt.append(q)
                return qt

            def emit_scores(hp, qt):
                # e-main: per (c, hh) one [64,128]x[64,512] matmul -> exp.
                ex0 = []
                for c in range(NCOMP):
                    ex = bc.tile([128, 2 * SL], BF16, tag="ex0", bufs=8,
                                 name=f"ex0_{hp}_{c}")
                    for hh in range(2):
                        ps = bcp.tile([128, SL], F32, tag="sc", bufs=4,
                                      name=f"ps0_{hp}_{c}_{hh}")
                        nc.tensor.matmul(
                            ps[:],
                            kt_sb[hp][hh * 64:(hh + 1) * 64,
                                      c * 128:c * 128 + 128],
                            qt[c][hh * 64:(hh + 1) * 64, :],
                            start=True, stop=True)
                        nc.scalar.activation(
                            ex[:, hh * SL:(hh + 1) * SL], ps[:], AF.Exp)
                    ex0.append(ex)
                # e-tail: two comps per tile at partition offsets 0/64.
                ex1 = []
                for g in range(2):
                    ex = bc.tile([128, 2 * SL], BF16, tag="ex1", bufs=4,
                                 name=f"ex1_{hp}_{g}")
                    for hh in range(2):
                        ps1 = bcp.tile([128, SL], F32, tag="sc", bufs=4,
                                       name=f"ps1_{hp}_{g}_{hh}")
                        for cc in range(2):
                            c = 2 * g + cc
                            nc.tensor.matmul(
                                ps1[cc * 64:(cc + 1) * 64, :],
                                kt_sb[hp][hh * 64:(hh + 1) * 64,
                                          512 + c * 64:512 + c * 64 + 64],
                                qt[c][hh * 64:(hh + 1) * 64, :],
                                start=True, stop=True, skip_group_check=True)
                        nc.scalar.activation(
                            ex[:, hh * SL:(hh + 1) * SL], ps1[:], AF.Exp)
                    ex1.append(ex)
                return ex0, ex1

            def emit_softmax(hp, ex0, ex1):
                # e-main: sum over comps, reciprocal, normalize in place.
                ssum = bc.tile([128, 2 * SL], F32, tag="ssum", bufs=2,
                               name=f"sm{hp}")
                nc.vector.tensor_add(out=ssum[:], in0=ex0[0][:], in1=ex0[1][:])
                nc.vector.tensor_add(out=ssum[:], in0=ssum[:], in1=ex0[2][:])
                nc.vector.tensor_add(out=ssum[:], in0=ssum[:], in1=ex0[3][:])
                rinv = bc.tile([128, 2 * SL], F32, tag="rinv", bufs=2,
                               name=f"ri{hp}")
                nc.vector.reciprocal_approx_fast(out=rinv[:], in_=ssum[:])
                for c in range(NCOMP):
                    nc.vector.tensor_mul(out=ex0[c][:], in0=ex0[c][:],
                                         in1=rinv[:])
                # e-tail: comp-sum via stacked-identity matmul (PE); the sum
                # lands replicated at both partition offsets.
                rinv1 = bc.tile([128, 2 * SL], F32, tag="rinv1", bufs=2,
                                name=f"ri1_{hp}")
                for hh in range(2):
                    pss = bcp.tile([128, SL], F32, tag="sc", bufs=4,
                                   name=f"pss{hp}_{hh}")
                    nc.tensor.matmul(
                        pss[:], ist_sb[:], ex1[0][:, hh * SL:(hh + 1) * SL],
                        start=True, stop=False)
                    nc.tensor.matmul(
                        pss[:], ist_sb[:], ex1[1][:, hh * SL:(hh + 1) * SL],
                        start=False, stop=True)
                    nc.vector.reciprocal_approx_fast(
                        out=rinv1[:, hh * SL:(hh + 1) * SL], in_=pss[:])
                for g in range(2):
                    nc.vector.tensor_mul(out=ex1[g][:], in0=ex1[g][:],
                                         in1=rinv1[:])

            def emit_o(hp, ex0, ex1):
                for c in range(NCOMP):
                    g, cc = divmod(c, 2)
                    po = bcp.tile([128, SL], F32, tag="po", bufs=2,
                                  name=f"po{hp}_{c}")
                    for hh in range(2):
                        h = hp * 2 + hh
                        nc.tensor.matmul(
                            po[hh * 64:(hh + 1) * 64, :],
                            v_sb[c][:, h * 64:(h + 1) * 64],
                            ex0[c][:, hh * SL:(hh + 1) * SL],
                            start=True, stop=False, skip_group_check=True)
                        nc.tensor.matmul(
                            po[hh * 64:(hh + 1) * 64, :],
                            vpad[g][cc * 64:(cc + 1) * 64, h * 64:(h + 1) * 64],
                            ex1[g][cc * 64:(cc + 1) * 64,
                                   hh * SL:(hh + 1) * SL],
                            start=False, stop=True, skip_group_check=True)
                    ot = bc.tile([128, SL], BF16, tag="oT", bufs=48,
                                 name=f"ot{hp}_{c}")
                    nc.vector.tensor_copy(out=ot[:], in_=po[:])
                    ot_sb[(c, hp)] = ot

            def emit_c(fot, fi_hold=None):
                """Out-projection for feature block fot. If fi_hold is set,
                returns a closure finishing the first pass (the held fi step
                plus the second pair pass), so the caller can emit other work
                in between."""
                wo_b = bc.tile([128, FT * 128], BF16, tag="wo", bufs=2,
                               name=f"wo{fot}")
                nc.sync.dma_start(
                    out=wo_b.rearrange("p (f o) -> p f o", f=FT),
                    in_=woT_v[:, :, fot * 128:(fot + 1) * 128])

                def run_pass(pair, fis, pc=None, final=True):
                    cs = (2 * pair, 2 * pair + 1)
                    if pc is None:
                        pc = {c: bcp.tile([128, SL], F32, tag="pq", bufs=2,
                                          name=f"pc{fot}_{c}") for c in cs}
                    for fi in fis:
                        w_sl = wo_b[:, fi * 128:(fi + 1) * 128]
                        for c in cs:
                            nc.tensor.matmul(
                                pc[c][:], w_sl, ot_sb[(c, fi)][:],
                                start=(fi == 0),
                                stop=(final and fi == fis[-1]))
                    if not final:
                        return pc
                    for c in cs:
                        ob = bc.tile([128, SL], F32, tag="ob", bufs=3,
                                     name=f"ob{fot}_{c}")
                        nc.vector.scalar_tensor_tensor(
                            out=ob[:], in0=pc[c][:],
                            scalar=bo_sb[:, fot:fot + 1],
                            in1=xh[c][:, fot * SL:(fot + 1) * SL],
                            op0=ALU.add, op1=ALU.add)
                        nc.sync.dma_start(
                            out=outT_v[c][:, fot, :], in_=ob[:])
                    return None

                if fi_hold is None:
                    run_pass(0, list(range(FT)))
                    run_pass(1, list(range(FT)))
                    return None

                head = [fi for fi in range(FT) if fi != fi_hold]
                pc0 = run_pass(0, head, final=False)

                def finish():
                    run_pass(0, [fi_hold], pc=pc0)
                    run_pass(1, list(range(FT)))
                return finish

            # -------- pipelined emission --------
            if "B" in phases:
                prev = None
                for hp in range(HP):
                    qt = emit_q(hp)
                    if prev is not None:
                        emit_softmax(*prev)
                        emit_o(prev[0], prev[1], prev[2])
                    ex0, ex1 = emit_scores(hp, qt)
                    prev = (hp, ex0, ex1)
                # Tail: overlap the last softmax/o with the start of C.
                if "C" in phases:
                    finish = emit_c(0, fi_hold=HP - 1)
                    emit_softmax(*prev)
                    emit_o(prev[0], prev[1], prev[2])
                    finish()
                    for fot in range(1, FT):
                        emit_c(fot)
                else:
                    emit_softmax(*prev)
                    emit_o(prev[0], prev[1], prev[2])
            elif "C" in phases:
                for fot in range(FT):
                    emit_c(fot)


_NC_CACHE = {}


def _get_nc():
    if "nc" not in _NC_CACHE:
        nc = bacc.Bacc("TRN2", target_bir_lowering=False)
        with tile.TileContext(nc) as tc:
            _emit(tc)
        nc.compile()
        _NC_CACHE["nc"] = nc
    return _NC_CACHE["nc"]


def kernel(hidden_states, encoder_hidden_states, temperature, Wq, Wk, Wv, Wo,
           bo, pad_length):
    # pad branch contributes zero to the output (zeros projected with no
    # bias give k_pad = v_pad = 0), so pad_length is irrelevant.
    hs = np.ascontiguousarray(np.asarray(hidden_states, dtype=np.float32))
    ehs = np.ascontiguousarray(
        np.asarray(encoder_hidden_states, dtype=np.float32))
    temp = float(np.asarray(temperature).reshape(-1)[0])
    Wq = np.asarray(Wq, dtype=np.float32)
    Wk = np.asarray(Wk, dtype=np.float32)
    Wv = np.asarray(Wv, dtype=np.float32)
    Wo = np.asarray(Wo, dtype=np.float32)
    bo_v = np.asarray(bo, dtype=np.float32).reshape(-1)

    wqT = np.ascontiguousarray((Wq / (temp + EPS)).T).astype(ml_dtypes.bfloat16)
    wkT = np.ascontiguousarray(Wk.T).astype(ml_dtypes.bfloat16)
    wvT = np.ascontiguousarray(Wv.T).astype(ml_dtypes.bfloat16)
    woT = np.ascontiguousarray(Wo.T).astype(ml_dtypes.bfloat16)
    # Column layout: [c0 e0:128 | c1 | c2 | c3 | c0 e128:154+pad | c1 | c2 | c3]
    eT_all = np.zeros((D, ECAT), dtype=ml_dtypes.bfloat16)
    for c in range(NCOMP):
        ecT = ehs[c].T.astype(ml_dtypes.bfloat16)
        eT_all[:, c * 128:c * 128 + 128] = ecT[:, 0:128]
        eT_all[:, 512 + c * 64:512 + c * 64 + (E - 128)] = ecT[:, 128:E]
    bo_t = np.ascontiguousarray(bo_v.reshape(FT, 128).T)
    # ist[k, m] = 1 iff k % 64 == m % 64: comp-sum over partition groups,
    # replicated to both 64-row offsets.
    ist = np.tile(np.eye(64, dtype=np.float32), (2, 2)).astype(
        ml_dtypes.bfloat16)

    nc = _get_nc()
    in_maps = []
    for i in range(NCORES):
        xTb_i = np.ascontiguousarray(
            hs[:, i * SL:(i + 1) * SL, :].transpose(0, 2, 1)).astype(
                ml_dtypes.bfloat16)
        in_maps.append({
            "xTb": xTb_i, "eT": eT_all, "wqT": wqT, "wkT": wkT,
            "wvT": wvT, "woT": woT, "bo": bo_t, "ist": ist,
        })

    res = run_bass_kernel_spmd(nc, in_maps, core_ids=list(range(NCORES)))

    out = np.empty((NCOMP, S, D), dtype=np.float32)
    for i in range(NCORES):
        out[:, i * SL:(i + 1) * SL, :] = res.results[i]["outT"].transpose(
            0, 2, 1)
    return out


# revision 7
# speedup vs baseline: 1.9878x; 1.0726x over previous
"""Trainium2 Bass kernel for DecomposingAttnProcessor (pad variant).

Math (pad branch contributes exactly zero since pad tokens are zeros
projected with no bias -> k_pad = v_pad = 0):
    q = hs @ Wq.T / (temp + eps)   (scale folded into Wq on host)
    k = ehs @ Wk.T ; v = ehs @ Wv.T
    scores[c,h,s,e] = q . k        (per head, dh=64)
    w = softmax over the 4 components c (dim 0)
    o = w @ v ; out = o @ Wo.T + bo + hs

Sharding: 8 cores, split S=4096 into 512-row blocks; all 4 components of
a block stay on one core (softmax couples them). K/V computed redundantly
per core (encoder seq is only 154).

Layout notes:
- Device layout fully transposed (features on partitions).
- E padded to 192 host-side with zeros: padded e slots give scores=0 ->
  exp(0)=1 -> w=1/4, but v rows there are 0, so output unaffected.
- Per component: e split (0:128) in v_sb[c]; the 64-row e-tails are packed
  two components per tile (partition offsets 0/64): vpad[0]=(c0,c1),
  vpad[1]=(c2,c3). Same packing for the tail scores/weights (ex1 tiles).
- Softmax over the 4 components: e-main via 3 DVE adds; e-tail comp-sum
  via a stacked-identity matmul on PE (result replicated to both offsets).
- Single s-pass of 512 (N=512 matmuls), weights streamed once; residual
  comes from the bf16 x panels already in SBUF.
"""

import numpy as np
import ml_dtypes

import concourse.bass as bass
import concourse.mybir as mybir
import concourse.tile as tile
from concourse import bacc
from concourse.bass_utils import run_bass_kernel_spmd

F32 = mybir.dt.float32
BF16 = mybir.dt.bfloat16
AF = mybir.ActivationFunctionType
ALU = mybir.AluOpType

NCOMP = 4
HEADS = 24
DH = 64
D = 1536
S = 4096
E = 154
EP = 192                  # e padded so the tail is 64 rows (offsets 0/64 only)
ECAT = NCOMP * EP         # 768
EPS = 1e-8
NCORES = 8
SL = S // NCORES          # 512 s-rows per core (per component)
FT = D // 128             # 12 feature tiles of 128
HP = HEADS // 2           # 12 head-pairs (2 heads = 128 feature rows)


def _emit(tc):
    import os
    phases = os.environ.get("K_PHASES", "ABC")
    nc = tc.nc

    xTb = nc.declare_dram_parameter("xTb", [NCOMP, D, SL], BF16, isOutput=False)
    eT = nc.declare_dram_parameter("eT", [D, ECAT], BF16, isOutput=False)
    wqT = nc.declare_dram_parameter("wqT", [D, D], BF16, isOutput=False)
    wkT = nc.declare_dram_parameter("wkT", [D, D], BF16, isOutput=False)
    wvT = nc.declare_dram_parameter("wvT", [D, D], BF16, isOutput=False)
    woT = nc.declare_dram_parameter("woT", [D, D], BF16, isOutput=False)
    bo = nc.declare_dram_parameter("bo", [128, FT], F32, isOutput=False)
    ist = nc.declare_dram_parameter("ist", [128, 128], BF16, isOutput=False)
    outT = nc.declare_dram_parameter("outT", [NCOMP, D, SL], F32, isOutput=True)

    # DRAM views with the 128-row tile index folded into the free dim, so a
    # whole [1536, n] panel loads as one DMA into a [128, FT*n] tile.
    xTb_v = [xTb[c].rearrange("(f p) s -> p f s", p=128) for c in range(NCOMP)]
    eT_v = eT.rearrange("(f p) e -> p f e", p=128)
    wqT_v = wqT.rearrange("(f p) o -> p f o", p=128)
    wkT_v = wkT.rearrange("(f p) o -> p f o", p=128)
    wvT_v = wvT.rearrange("(f p) o -> p f o", p=128)
    woT_v = woT.rearrange("(f p) o -> p f o", p=128)
    outT_v = [outT[c].rearrange("(f p) s -> p f s", p=128) for c in range(NCOMP)]

    with tc.tile_pool(name="persist", bufs=1) as pp:
        # ---------------- persistent tiles ----------------
        kt_sb = [pp.tile([128, ECAT], BF16, tag="kT", bufs=FT, name=f"kt{t}")
                 for t in range(FT)]
        v_sb = [pp.tile([128, D], BF16, tag="v", bufs=NCOMP, name=f"v{c}")
                for c in range(NCOMP)]
        vpad = [pp.tile([128, D], BF16, tag="vpad", bufs=2, name=f"vpad{g}")
                for g in range(2)]
        bo_sb = pp.tile([128, FT], F32, tag="bo", bufs=1, name="bo_sb")
        ist_sb = pp.tile([128, 128], BF16, tag="ist", bufs=1, name="ist_sb")
        xh = [pp.tile([128, FT * SL], BF16, tag="xh", bufs=NCOMP, name=f"xh{c}")
              for c in range(NCOMP)]
        nc.sync.dma_start(out=bo_sb[:], in_=bo[:])
        nc.sync.dma_start(out=ist_sb[:], in_=ist[:])

        # ---------------- phase A: K^T and V ----------------
        with (
            tc.tile_pool(name="pha", bufs=1) as pa,
            tc.tile_pool(name="pap", bufs=1, space="PSUM") as pap,
        ):
            # e panels in 3 chunks of 4 feature-tiles so the first K^T
            # matmuls can start after ~1/3 of the transfer.
            et_ch = [pa.tile([128, 4 * ECAT], BF16, tag="eT", bufs=3,
                             name=f"et{ch}") for ch in range(3)]
            for ch in range(3):
                nc.sync.dma_start(
                    out=et_ch[ch].rearrange("p (f e) -> p f e", f=4),
                    in_=eT_v[:, 4 * ch:4 * (ch + 1), :])

            class _Et:
                def __getitem__(self, key):
                    _, fi, esl = key
                    return et_ch[fi // 4][:, (fi % 4) * ECAT + esl.start:
                                          (fi % 4) * ECAT + esl.stop]
            et = _Et()

            if "A" in phases:
                # K^T[fo, c*EP + e] accumulated over fi; N split 512+256.
                for fot in range(FT):
                    wk_b = pa.tile([128, FT * 128], BF16, tag="wk", bufs=3,
                                   name=f"wk{fot}")
                    nc.sync.dma_start(
                        out=wk_b.rearrange("p (f o) -> p f o", f=FT),
                        in_=wkT_v[:, :, fot * 128:(fot + 1) * 128])
                    pk0 = pap.tile([128, 512], F32, tag="pk0", bufs=2,
                                   name=f"pk0_{fot}")
                    pk1 = pap.tile([128, 256], F32, tag="pk1", bufs=2,
                                   name=f"pk1_{fot}")
                    for fi in range(FT):
                        w_sl = wk_b[:, fi * 128:(fi + 1) * 128]
                        nc.tensor.matmul(pk0[:], w_sl, et[:, fi, 0:512],
                                         start=(fi == 0), stop=(fi == FT - 1))
                        nc.tensor.matmul(pk1[:], w_sl, et[:, fi, 512:768],
                                         start=(fi == 0), stop=(fi == FT - 1))
                    nc.vector.tensor_copy(out=kt_sb[fot][:, 0:512], in_=pk0[:])
                    nc.vector.tensor_copy(out=kt_sb[fot][:, 512:768], in_=pk1[:])

                # V[c][e, fv] (natural layout, bf16) accumulated over fi.
                for fvc in range(3):
                    wv_b = pa.tile([128, FT * 512], BF16, tag="wv", bufs=2,
                                   name=f"wv{fvc}")
                    nc.sync.dma_start(
                        out=wv_b.rearrange("p (f o) -> p f o", f=FT),
                        in_=wvT_v[:, :, fvc * 512:(fvc + 1) * 512])
                    if fvc == 0:
                        # x panels: needed only at phase-B start; issued here
                        # so they don't delay the phase-A weight streams.
                        for c in range(NCOMP):
                            nc.sync.dma_start(
                                out=xh[c].rearrange("p (f s) -> p f s", f=FT),
                                in_=xTb_v[c])
                    for c in range(NCOMP):
                        pv = pap.tile([128, 512], F32, tag="pv", bufs=2,
                                      name=f"pv{fvc}_{c}")
                        for fi in range(FT):
                            nc.tensor.matmul(
                                pv[:],
                                et[:, fi, c * 128:c * 128 + 128],
                                wv_b[:, fi * 512:(fi + 1) * 512],
                                start=(fi == 0), stop=(fi == FT - 1))
                        nc.vector.tensor_copy(
                            out=v_sb[c][:, fvc * 512:(fvc + 1) * 512], in_=pv[:])
                    # e-tail rows, two comps per tile at offsets 0/64.
                    for g in range(2):
                        pvp = pap.tile([128, 512], F32, tag="pv", bufs=2,
                                       name=f"pvp{fvc}_{g}")
                        for fi in range(FT):
                            nc.tensor.matmul(
                                pvp[:],
                                et[:, fi, 512 + g * 128:512 + g * 128 + 128],
                                wv_b[:, fi * 512:(fi + 1) * 512],
                                start=(fi == 0), stop=(fi == FT - 1))
                        nc.vector.tensor_copy(
                            out=vpad[g][:, fvc * 512:(fvc + 1) * 512],
                            in_=pvp[:])

        # ---------------- phases B+C ----------------
        with (
            tc.tile_pool(name="bc", bufs=1) as bc,
            tc.tile_pool(name="bcp", bufs=1, space="PSUM") as bcp,
        ):
            ot_sb = {}

            def emit_q(hp):
                wq_b = bc.tile([128, FT * 128], BF16, tag="wq", bufs=2,
                               name=f"wq{hp}")
                nc.sync.dma_start(
                    out=wq_b.rearrange("p (f o) -> p f o", f=FT),
                    in_=wqT_v[:, :, hp * 128:(hp + 1) * 128])
                qt = []
                for pair in range(2):
                    cs = (2 * pair, 2 * pair + 1)
                    pq = {c: bcp.tile([128, SL], F32, tag="pq", bufs=2,
                                      name=f"pq{hp}_{c}") for c in cs}
                    for fi in range(FT):
                        w_sl = wq_b[:, fi * 128:(fi + 1) * 128]
                        for c in cs:
                            nc.tensor.matmul(
                                pq[c][:], w_sl, xh[c][:, fi * SL:(fi + 1) * SL],
                                start=(fi == 0), stop=(fi == FT - 1))
                    for c in cs:
                        q = bc.tile([128, SL], BF16, tag="qT", bufs=6,
                                    name=f"qt{hp}_{c}")
                        nc.vector.tensor_copy(out=q[:], in_=pq[c][:])
                        qt.append(q)
                return qt

            def emit_scores(hp, qt):
                # e-main: per (c, hh) one [64,128]x[64,512] matmul -> exp.
                ex0 = []
                for c in range(NCOMP):
                    ex = bc.tile([128, 2 * SL], BF16, tag="ex0", bufs=8,
                                 name=f"ex0_{hp}_{c}")
                    for hh in range(2):
                        ps = bcp.tile([128, SL], F32, tag="sc", bufs=5,
                                      name=f"ps0_{hp}_{c}_{hh}")
                        nc.tensor.matmul(
                            ps[:],
                            kt_sb[hp][hh * 64:(hh + 1) * 64,
                                      c * 128:c * 128 + 128],
                            qt[c][hh * 64:(hh + 1) * 64, :],
                            start=True, stop=True)
                        nc.scalar.activation(
                            ex[:, hh * SL:(hh + 1) * SL], ps[:], AF.Exp)
                    ex0.append(ex)
                # e-tail: two comps per tile at partition offsets 0/64.
                ex1 = []
                for g in range(2):
                    ex = bc.tile([128, 2 * SL], BF16, tag="ex1", bufs=4,
                                 name=f"ex1_{hp}_{g}")
                    for hh in range(2):
                        ps1 = bcp.tile([128, SL], F32, tag="sc", bufs=5,
                                       name=f"ps1_{hp}_{g}_{hh}")
                        for cc in range(2):
                            c = 2 * g + cc
                            nc.tensor.matmul(
                                ps1[cc * 64:(cc + 1) * 64, :],
                                kt_sb[hp][hh * 64:(hh + 1) * 64,
                                          512 + c * 64:512 + c * 64 + 64],
                                qt[c][hh * 64:(hh + 1) * 64, :],
                                start=True, stop=True, skip_group_check=True)
                        nc.scalar.activation(
                            ex[:, hh * SL:(hh + 1) * SL], ps1[:], AF.Exp)
                    ex1.append(ex)
                return ex0, ex1

            def emit_softmax(hp, ex0, ex1):
                # e-main: sum over comps, reciprocal, normalize in place.
                ssum = bc.tile([128, 2 * SL], F32, tag="ssum", bufs=2,
                               name=f"sm{hp}")
                nc.vector.tensor_add(out=ssum[:], in0=ex0[0][:], in1=ex0[1][:])
                nc.vector.tensor_add(out=ssum[:], in0=ssum[:], in1=ex0[2][:])
                nc.vector.tensor_add(out=ssum[:], in0=ssum[:], in1=ex0[3][:])
                rinv = bc.tile([128, 2 * SL], F32, tag="rinv", bufs=2,
                               name=f"ri{hp}")
                nc.vector.reciprocal_approx_fast(out=rinv[:], in_=ssum[:])
                for c in range(NCOMP):
                    nc.vector.tensor_mul(out=ex0[c][:], in0=ex0[c][:],
                                         in1=rinv[:])
                # e-tail: comp-sum via stacked-identity matmul (PE); the sum
                # lands replicated at both partition offsets.
                rinv1 = bc.tile([128, 2 * SL], F32, tag="rinv1", bufs=2,
                                name=f"ri1_{hp}")
                for hh in range(2):
                    pss = bcp.tile([128, SL], F32, tag="sc", bufs=5,
                                   name=f"pss{hp}_{hh}")
                    nc.tensor.matmul(
                        pss[:], ist_sb[:], ex1[0][:, hh * SL:(hh + 1) * SL],
                        start=True, stop=False)
                    nc.tensor.matmul(
                        pss[:], ist_sb[:], ex1[1][:, hh * SL:(hh + 1) * SL],
                        start=False, stop=True)
                    nc.vector.reciprocal_approx_fast(
                        out=rinv1[:, hh * SL:(hh + 1) * SL], in_=pss[:])
                for g in range(2):
                    nc.vector.tensor_mul(out=ex1[g][:], in0=ex1[g][:],
                                         in1=rinv1[:])

            def emit_o(hp, ex0, ex1):
                for c in range(NCOMP):
                    g, cc = divmod(c, 2)
                    po = bcp.tile([128, SL], F32, tag="po", bufs=1,
                                  name=f"po{hp}_{c}")
                    for hh in range(2):
                        h = hp * 2 + hh
                        nc.tensor.matmul(
                            po[hh * 64:(hh + 1) * 64, :],
                            v_sb[c][:, h * 64:(h + 1) * 64],
                            ex0[c][:, hh * SL:(hh + 1) * SL],
                            start=True, stop=False, skip_group_check=True)
                        nc.tensor.matmul(
                            po[hh * 64:(hh + 1) * 64, :],
                            vpad[g][cc * 64:(cc + 1) * 64, h * 64:(h + 1) * 64],
                            ex1[g][cc * 64:(cc + 1) * 64,
                                   hh * SL:(hh + 1) * SL],
                            start=False, stop=True, skip_group_check=True)
                    ot = bc.tile([128, SL], BF16, tag="oT", bufs=48,
                                 name=f"ot{hp}_{c}")
                    nc.vector.tensor_copy(out=ot[:], in_=po[:])
                    ot_sb[(c, hp)] = ot

            def load_wo(fot):
                wo_b = bc.tile([128, FT * 128], BF16, tag="wo", bufs=3,
                               name=f"wo{fot}")
                nc.sync.dma_start(
                    out=wo_b.rearrange("p (f o) -> p f o", f=FT),
                    in_=woT_v[:, :, fot * 128:(fot + 1) * 128])
                return wo_b

            def emit_c(fot, fi_hold=None, wo_pre=None):
                """Out-projection for feature block fot. If fi_hold is set,
                returns a closure finishing the first pass (the held fi step
                plus the second pair pass), so the caller can emit other work
                in between."""
                wo_b = wo_pre if wo_pre is not None else load_wo(fot)

                def run_pass(pair, fis, pc=None, final=True):
                    cs = (2 * pair, 2 * pair + 1)
                    tag = "pq" if pair == 0 else "sc"
                    if pc is None:
                        pc = {c: bcp.tile([128, SL], F32, tag=tag,
                                          bufs=2 if pair == 0 else 5,
                                          name=f"pc{fot}_{c}") for c in cs}
                    for fi in fis:
                        w_sl = wo_b[:, fi * 128:(fi + 1) * 128]
                        for c in cs:
                            nc.tensor.matmul(
                                pc[c][:], w_sl, ot_sb[(c, fi)][:],
                                start=(fi == 0),
                                stop=(final and fi == fis[-1]))
                    if not final:
                        return pc
                    for c in cs:
                        ob = bc.tile([128, SL], F32, tag="ob", bufs=3,
                                     name=f"ob{fot}_{c}")
                        nc.vector.scalar_tensor_tensor(
                            out=ob[:], in0=pc[c][:],
                            scalar=bo_sb[:, fot:fot + 1],
                            in1=xh[c][:, fot * SL:(fot + 1) * SL],
                            op0=ALU.add, op1=ALU.add)
                        nc.sync.dma_start(
                            out=outT_v[c][:, fot, :], in_=ob[:])
                    return None

                if fi_hold is None:
                    run_pass(0, list(range(FT)))
                    run_pass(1, list(range(FT)))
                    return None

                head = [fi for fi in range(FT) if fi != fi_hold]
                pc0 = run_pass(0, head, final=False)

                def finish():
                    run_pass(0, [fi_hold], pc=pc0)
                    run_pass(1, list(range(FT)))
                return finish

            # -------- pipelined emission --------
            if "B" in phases:
                prev = None
                wo0 = None
                for hp in range(HP):
                    qt = emit_q(hp)
                    if prev is not None:
                        emit_softmax(*prev)
                        emit_o(prev[0], prev[1], prev[2])
                    ex0, ex1 = emit_scores(hp, qt)
                    prev = (hp, ex0, ex1)
                    if hp == HP - 2 and "C" in phases:
                        wo0 = load_wo(0)
                # Tail: overlap the last softmax/o with the start of C.
                if "C" in phases:
                    finish = emit_c(0, fi_hold=HP - 1, wo_pre=wo0)
                    emit_softmax(*prev)
                    emit_o(prev[0], prev[1], prev[2])
                    finish()
                    for fot in range(1, FT):
                        emit_c(fot)
                else:
                    emit_softmax(*prev)
                    emit_o(prev[0], prev[1], prev[2])
            elif "C" in phases:
                for fot in range(FT):
                    emit_c(fot)


_NC_CACHE = {}


def _get_nc():
    if "nc" not in _NC_CACHE:
        nc = bacc.Bacc("TRN2", target_bir_lowering=False)
        with tile.TileContext(nc) as tc:
            _emit(tc)
        nc.compile()
        _NC_CACHE["nc"] = nc
    return _NC_CACHE["nc"]


def kernel(hidden_states, encoder_hidden_states, temperature, Wq, Wk, Wv, Wo,
           bo, pad_length):
    # pad branch contributes zero to the output (zeros projected with no
    # bias give k_pad = v_pad = 0), so pad_length is irrelevant.
    hs = np.ascontiguousarray(np.asarray(hidden_states, dtype=np.float32))
    ehs = np.ascontiguousarray(
        np.asarray(encoder_hidden_states, dtype=np.float32))
    temp = float(np.asarray(temperature).reshape(-1)[0])
    Wq = np.asarray(Wq, dtype=np.float32)
    Wk = np.asarray(Wk, dtype=np.float32)
    Wv = np.asarray(Wv, dtype=np.float32)
    Wo = np.asarray(Wo, dtype=np.float32)
    bo_v = np.asarray(bo, dtype=np.float32).reshape(-1)

    wqT = np.ascontiguousarray((Wq / (temp + EPS)).T).astype(ml_dtypes.bfloat16)
    wkT = np.ascontiguousarray(Wk.T).astype(ml_dtypes.bfloat16)
    wvT = np.ascontiguousarray(Wv.T).astype(ml_dtypes.bfloat16)
    woT = np.ascontiguousarray(Wo.T).astype(ml_dtypes.bfloat16)
    # Column layout: [c0 e0:128 | c1 | c2 | c3 | c0 e128:154+pad | c1 | c2 | c3]
    eT_all = np.zeros((D, ECAT), dtype=ml_dtypes.bfloat16)
    for c in range(NCOMP):
        ecT = ehs[c].T.astype(ml_dtypes.bfloat16)
        eT_all[:, c * 128:c * 128 + 128] = ecT[:, 0:128]
        eT_all[:, 512 + c * 64:512 + c * 64 + (E - 128)] = ecT[:, 128:E]
    bo_t = np.ascontiguousarray(bo_v.reshape(FT, 128).T)
    # ist[k, m] = 1 iff k % 64 == m % 64: comp-sum over partition groups,
    # replicated to both 64-row offsets.
    ist = np.tile(np.eye(64, dtype=np.float32), (2, 2)).astype(
        ml_dtypes.bfloat16)

    nc = _get_nc()
    in_maps = []
    for i in range(NCORES):
        xTb_i = np.ascontiguousarray(
            hs[:, i * SL:(i + 1) * SL, :].transpose(0, 2, 1)).astype(
                ml_dtypes.bfloat16)
        in_maps.append({
            "xTb": xTb_i, "eT": eT_all, "wqT": wqT, "wkT": wkT,
            "wvT": wvT, "woT": woT, "bo": bo_t, "ist": ist,
        })

    res = run_bass_kernel_spmd(nc, in_maps, core_ids=list(range(NCORES)))

    out = np.empty((NCOMP, S, D), dtype=np.float32)
    for i in range(NCORES):
        out[:, i * SL:(i + 1) * SL, :] = res.results[i]["outT"].transpose(
            0, 2, 1)
    return out
